# revision 9
# baseline (speedup 1.0000x reference)
"""AWQ 4-bit quantized linear (nn_AWQLinear) on 8 Trainium2 NeuronCores.

out[b,s,o] = fp16(sum_k x[b,s,k] * w[o,k]) + bias[o]
w[o,k] = (q[o,k] - z[o,k//128]) * s[o,k//128],  q packed 8 nibbles / int32.

Sharding: column-parallel (per spec hint). qweight/qzeros/scales/bias are
split along O=11008 into 8 shards of 1376; x is replicated; per-core
[4096, 1376] outputs are concatenated on host.

v4 layout — quad-packed dequant, per-slice weight tiles:
  K is processed in 32 chunks of 128; chunk kt = (Q, t) with Q = kt//4 a
  "quad" of 4 consecutive k-groups and t = kt%4 a nibble index. Partition
  p = j*32 + c of chunk (Q, t) holds original k = (4Q+j)*128 + 4c + t, so
  ONE [128, 1376] u16 tile ("qwq", halfword c of group 4Q+j at column o,
  host-gathered, unique bytes only) serves all four nibble extractions:
    slice t:  u32-bitcast AND with packed mask (DVE tensor_scalar, halves
              the column count; mask tiles built by memset, no DMA)
              -> ScalarE activation Copy(scale=2^-4t): u16 -> f16 nibble
              -> TT w = nib * s_b ; TT w -= zs_b  (f16, 2x tier;
                 gpsimd takes slice 3 of quads 0-5; the dequant tail
                 (quads 6-7) stays on DVE so gpsimd never gates pass B)
  Each W slice is its OWN tile so a chunk's matmuls depend only on that
  slice's two writers (no whole-quad false dependency). s_b/zs_b arrive
  as separate per-quad DMAs that block-replicate 4 scale rows 32x across
  partitions ([4g,1376] -> [128,1376], 0.35 MB each, 6-deep pools so the
  x-tile stream can't starve them); zs = z*s host-prepped. x is
  host-permuted to the chunk layout and pre-transposed to [K, M].

Matmul: psum [m=128, o<=512] accumulates 32 k-chunk matmuls (lhsT = xT
tile slice, rhs = W chunk slice). mb0 runs k-major: pass A fills all 8
psum banks (ms0,1 x 3 o-tiles + ms2 x 2) consuming ~6.3us/quad to ride
the dequant wave; pass B covers the remaining 4 tiles. Later m-blocks
run ms-outer with 3 banks and ob-inner so consecutive matmuls share the
stationary operand. Epilogue: single DVE tensor_tensor adds the
partition-replicated bias while copying PSUM->SBUF fp16; outputs ride
alternating scalar/sync HWDGE queues. s_b/zs_b/qweight/x all issue on
the sync queue; the scalar engine does only ACT converts during dequant.
"""

import sys

sys.path.insert(0, "/opt/trn_rl_repo")

import numpy as np

import concourse.bass as bass
import concourse.tile as tile
from concourse import bacc, mybir
from concourse import bass_utils

P = 128
N_CORES = 8
O_FULL = 11008
O_SHARD = O_FULL // N_CORES  # 1376
K = 4096
G = 32  # k-groups of 128
QUADS = G // 4  # 8
M = 4096  # tokens = 2*2048
M_TILE = 512
O_TILES = [(0, 512), (512, 512), (1024, O_SHARD - 1024)]  # (offset, width)

f16 = mybir.dt.float16
u16 = mybir.dt.uint16
u32 = mybir.dt.uint32
f32 = mybir.dt.float32


def build(n_mblocks=M // M_TILE, repeat=1):
    nc = bacc.Bacc("TRN2", target_bir_lowering=False, debug=False, num_devices=N_CORES)

    x_ap = nc.dram_tensor("xT", (K, M), f16, kind="ExternalInput").ap()
    qwq_ap = nc.dram_tensor("qwq", (QUADS, P, O_SHARD), u16, kind="ExternalInput").ap()
    szq_ap = nc.dram_tensor("szq", (QUADS, 4, 2, O_SHARD), f16, kind="ExternalInput").ap()
    bias_ap = nc.dram_tensor("bias", (1, O_SHARD), f16, kind="ExternalInput").ap()
    out_ap = nc.dram_tensor(
        "out", (n_mblocks * M_TILE, O_SHARD), f16, kind="ExternalOutput"
    ).ap()

    with tile.TileContext(nc) as tc:
      for _rep in range(repeat):
        with (
            tc.tile_pool(name="const", bufs=1) as const,
            tc.tile_pool(name="wt", bufs=4 * QUADS) as wt_pool,
            tc.tile_pool(name="deq", bufs=3) as deq,
            tc.tile_pool(name="xt", bufs=42) as xt_pool,
            tc.tile_pool(name="outp", bufs=6) as outp,
            tc.tile_pool(name="psum", bufs=8, space="PSUM") as psum,
        ):
            msk_sb = const.tile([P, 4], u32)
            for t in range(4):
                m = (0xF << (4 * t)) & 0xFFFF
                nc.gpsimd.memset(msk_sb[:, t : t + 1], (m << 16) | m)
            bias_rep = const.tile([P, O_SHARD], f16)
            nc.gpsimd.dma_start(
                out=bias_rep,
                in_=bass.AP(
                    tensor=bias_ap.tensor,
                    offset=bias_ap.offset,
                    ap=[[0, P], [1, O_SHARD]],
                ),
            )

            # per-chunk weight slices: WS[4q+t] = [128, 1376]
            WS = [
                wt_pool.tile([P, O_SHARD], f16, tag="wt", name=f"ws{_rep}_{kt}")
                for kt in range(G)
            ]

            # ---- dequant ----
            def issue_quad_dmas(q, eng):
                rt = deq.tile([P, O_SHARD], u16, tag="rt", bufs=6)
                eng.dma_start(out=rt, in_=qwq_ap[q])
                s_b = deq.tile([P, O_SHARD], f16, tag="sb", bufs=6)
                eng.dma_start(
                    out=s_b,
                    in_=bass.AP(
                        tensor=szq_ap.tensor,
                        offset=szq_ap.offset + q * 4 * 2 * O_SHARD,
                        ap=[[2 * O_SHARD, 4], [0, 32], [1, O_SHARD]],
                    ),
                )
                zs_b = deq.tile([P, O_SHARD], f16, tag="zb", bufs=6)
                eng.dma_start(
                    out=zs_b,
                    in_=bass.AP(
                        tensor=szq_ap.tensor,
                        offset=szq_ap.offset + (q * 4 * 2 + 1) * O_SHARD,
                        ap=[[2 * O_SHARD, 4], [0, 32], [1, O_SHARD]],
                    ),
                )
                return rt, s_b, zs_b

            # quads 0-5 issue on sync, interleaved with mb0 x tiles so
            # neither stream blocks the other; quads 6-7 issue from the
            # scalar queue mid-compute (their pool-recycle waits must not
            # stall the ACT chain or the x stream)
            xts0 = []
            quad_bufs = {}
            xt_sched = [4, 6, 6, 6, 6, 4]
            for q in range(6):
                quad_bufs[q] = issue_quad_dmas(q, nc.sync)
                for _ in range(xt_sched[q]):
                    g = len(xts0)
                    xtile = xt_pool.tile([P, M_TILE], f16, tag="xt", name="xt")
                    nc.sync.dma_start(
                        out=xtile, in_=x_ap[g * P : (g + 1) * P, 0:M_TILE]
                    )
                    xts0.append(xtile)

            for q in range(QUADS):
                if q == 2:
                    quad_bufs[6] = issue_quad_dmas(6, nc.scalar)
                if q == 3:
                    quad_bufs[7] = issue_quad_dmas(7, nc.scalar)
                rt, s_b, zs_b = quad_bufs[q]
                for t in range(4):
                    na = deq.tile([P, O_SHARD], u16, tag="na", bufs=3)
                    nc.vector.tensor_scalar(
                        out=na.bitcast(u32),
                        in0=rt.bitcast(u32),
                        scalar1=msk_sb[:, t : t + 1],
                        scalar2=None,
                        op0=mybir.AluOpType.bitwise_and,
                    )
                    nf = deq.tile([P, O_SHARD], f16, tag="nf", bufs=3)
                    nc.scalar.activation(
                        out=nf,
                        in_=na,
                        func=mybir.ActivationFunctionType.Copy,
                        scale=float(2.0 ** (-4 * t)),
                    )
                    wslice = WS[4 * q + t]
                    eng = nc.gpsimd if (t == 3 and q < 6) else nc.vector
                    eng.tensor_tensor(
                        out=wslice,
                        in0=nf,
                        in1=s_b,
                        op=mybir.AluOpType.mult,
                    )
                    eng.tensor_tensor(
                        out=wslice,
                        in0=wslice,
                        in1=zs_b,
                        op=mybir.AluOpType.subtract,
                    )

            while len(xts0) < G:
                g = len(xts0)
                xtile = xt_pool.tile([P, M_TILE], f16, tag="xt", name="xt")
                nc.sync.dma_start(
                    out=xtile, in_=x_ap[g * P : (g + 1) * P, 0:M_TILE]
                )
                xts0.append(xtile)

            # ---- matmul ----
            def finish_group(ps, mb, ms, o0, ow, qi=0):
                ot = outp.tile([P, 512], f16, tag="ot", name="ot")
                nc.vector.tensor_tensor(
                    out=ot[:, :ow],
                    in0=ps,
                    in1=bias_rep[:, o0 : o0 + ow],
                    op=mybir.AluOpType.add,
                )
                m0 = mb * M_TILE + ms * P
                qeng = nc.sync if qi % 2 else nc.scalar
                qeng.dma_start(
                    out=out_ap[m0 : m0 + P, o0 : o0 + ow], in_=ot[:, :ow]
                )

            def kmajor_pass(mb, xts, groups):
                # one psum bank per (o-tile, ms); k-major so PE consumes
                # each dequant slice as soon as it lands
                pss = []
                for _ in groups:
                    pst = psum.tile([P, 512], f32, tag="ps", name="ps")
                    pss.append(pst)
                for kt in range(G):
                    for i, (o0, ow, ms) in enumerate(groups):
                        nc.tensor.matmul(
                            pss[i][:, :ow],
                            lhsT=xts[kt][:, ms * P : (ms + 1) * P],
                            rhs=WS[kt][:, o0 : o0 + ow],
                            start=(kt == 0),
                            stop=(kt == G - 1),
                        )
                for i, (o0, ow, ms) in enumerate(groups):
                    finish_group(pss[i][:, :ow], mb, ms, o0, ow, qi=i)

            for mb in range(n_mblocks):
                if mb == 0:
                    xts = xts0
                else:
                    xts = []
                    for kt in range(G):
                        xtile = xt_pool.tile([P, M_TILE], f16, tag="xt", name="xt")
                        nc.sync.dma_start(
                            out=xtile,
                            in_=x_ap[
                                kt * P : (kt + 1) * P,
                                mb * M_TILE : (mb + 1) * M_TILE,
                            ],
                        )
                        xts.append(xtile)
                if mb == 0:
                    # dequant still streaming: pass A fills all 8 psum banks
                    # so the PE consumes each chunk as fast as it lands
                    kmajor_pass(0, xts, [(o0, ow, ms) for ms in (0, 1)
                                         for (o0, ow) in O_TILES]
                                        + [(0, 512, 2), (512, 512, 2)])
                    kmajor_pass(0, xts, [(1024, O_SHARD - 1024, 2)]
                                        + [(o0, ow, 3) for (o0, ow) in O_TILES])
                    continue
                for ms in range(M_TILE // P):
                    # 3 psum banks, kt-major, ob-inner: one LDWEIGHTS feeds
                    # the 3 o-tiles (redundant loads removed by ldw-opt)
                    pss = []
                    for _ in O_TILES:
                        pss.append(psum.tile([P, 512], f32, tag="ps", name="ps"))
                    for kt in range(G):
                        for i, (o0, ow) in enumerate(O_TILES):
                            nc.tensor.matmul(
                                pss[i][:, :ow],
                                lhsT=xts[kt][:, ms * P : (ms + 1) * P],
                                rhs=WS[kt][:, o0 : o0 + ow],
                                start=(kt == 0),
                                stop=(kt == G - 1),
                            )
                    for i, (o0, ow) in enumerate(O_TILES):
                        finish_group(pss[i][:, :ow], mb, ms, o0, ow, qi=i + ms)

    nc.compile()
    return nc


def _unpack_nib(a):
    shifts = (np.arange(8, dtype=np.int32) * 4).reshape(1, 1, 8)
    nib = (a[..., None] >> shifts) & 0xF
    return nib.reshape(a.shape[0], a.shape[1] * 8)


def make_in_maps(x, qweight, qzeros, scales, bias):
    # Chunk kt=(Q,t): partition p = j*32+c holds original k = (4Q+j)*128+4c+t.
    # Permute K accordingly and transpose to [K, M] so xT tiles load with
    # plain contiguous DMAs.
    x_flat = np.ascontiguousarray(
        x.reshape(M, QUADS, 4, 32, 4)      # [m, Q, j, c, t]
        .transpose(1, 4, 2, 3, 0)           # [Q, t, j, c, m]
        .reshape(K, M)
    )
    in_maps = []
    for i in range(N_CORES):
        sl = slice(i * O_SHARD, (i + 1) * O_SHARD)
        qw16 = np.ascontiguousarray(qweight[sl]).view(np.uint16)  # [O, 1024]
        # qwq[Q, j*32+c, o] = halfword (4Q+j)*32+c of row o
        qwq = np.ascontiguousarray(
            qw16.T.reshape(QUADS, 4 * 32, O_SHARD)
        )
        z = _unpack_nib(np.ascontiguousarray(qzeros[sl]))[:, :G].astype(np.float32)
        s = scales[sl, :G].astype(np.float32)
        zs = (z * s).astype(np.float16)  # [O, G]
        st = s.astype(np.float16)
        # szq[Q, g_in_quad, {s, zs}, o]
        szq = np.stack([st.T.reshape(G, O_SHARD), zs.T.reshape(G, O_SHARD)],
                       axis=1)               # [G, 2, O]
        szq = np.ascontiguousarray(szq.reshape(QUADS, 4, 2, O_SHARD))
        b = np.ascontiguousarray(bias[sl]).reshape(1, O_SHARD)
        in_maps.append(
            {"xT": x_flat, "qwq": qwq, "szq": szq, "bias": b}
        )
    return in_maps


_NC = None


def kernel(x, qweight, qzeros, scales, bias):
    global _NC
    x = np.asarray(x)
    qweight = np.asarray(qweight)
    qzeros = np.asarray(qzeros)
    scales = np.asarray(scales)
    bias = np.asarray(bias)
    if _NC is None:
        _NC = build()
    in_maps = make_in_maps(x, qweight, qzeros, scales, bias)
    res = bass_utils.run_bass_kernel_spmd(_NC, in_maps, core_ids=list(range(N_CORES)))
    shards = [res.results[i]["out"] for i in range(N_CORES)]
    out = np.concatenate(shards, axis=1).reshape(2, 2048, O_FULL)
    return out.astype(np.float16)
